# revision 11
# baseline (speedup 1.0000x reference)
"""Trainium2 Bass kernel: sliding-window multihead attention w/ ALiBi.

Computation (per reference):
  qkv = x @ w_in.T ; q,k,v heads ; blocked sliding-window causal attention
  (window=512, ALiBi bias slope_h*(q_idx-kv_idx)) ; out = o @ w_out.T

Sharding: 8 cores = 4 batches x 2 head-groups (8 heads each). Each core
computes its batch's QKV for its heads, attention, and a partial out-proj
over its heads' columns. Host sums the two head-group partials per batch.

Softmax trick: P = exp(s_raw) * EXPBIG where EXPBIG = exp(bias - bound)
is a host-precomputed Toeplitz band (exact 0 outside the valid window).
The row-max subtraction is replaced by a static bound folded into EXPBIG
(block 0 uses a per-partition ACT bias instead). The softmax denominator
comes from an appended ones-column in the V matmul; normalization uses a
K=2 broadcast matmul + vector reciprocal.
"""

import os
import numpy as np
from contextlib import ExitStack

import concourse.bass as bass
import concourse.bacc as bacc
import concourse.tile as tile
import concourse.mybir as mybir
from concourse.bass_utils import run_bass_kernel_spmd

F16 = mybir.dt.float16
F32 = mybir.dt.float32
AF = mybir.ActivationFunctionType
ALU = mybir.AluOpType

B, S, E = 4, 2048, 1024
H, D, WIN = 16, 64, 512
NB = S // WIN          # 4 blocks
HPC = 8                # heads per core
NCORES = 8
CM = 6.0               # softmax bound safety margin

LAST_RESULTS = None


def _qrange(jt):
    # valid q-column range for scores j-tile jt (window band)
    lo = max(0, 128 * jt - 512)
    hi = min(512, 128 * jt + 128)
    return lo, hi - lo


def _build_nc():
    nc = bacc.Bacc("TRN2", target_bir_lowering=False, debug=False,
                   num_devices=NCORES)

    xT = nc.dram_tensor("xT", [E, S], F16, kind="ExternalInput").ap()
    wqk = nc.dram_tensor("w_qk", [E, 1024], F16, kind="ExternalInput").ap()
    wv = nc.dram_tensor("w_v", [E, 512], F16, kind="ExternalInput").ap()
    wo = nc.dram_tensor("w_o", [512, E], F16, kind="ExternalInput").ap()
    ebig = nc.dram_tensor("expbig", [HPC, 128, 1408], F16,
                          kind="ExternalInput").ap()
    em0 = nc.dram_tensor("em0", [128, 512], F16, kind="ExternalInput").ap()
    b0v = nc.dram_tensor("b0v", [128, 32], F32, kind="ExternalInput").ap()
    ones2 = nc.dram_tensor("ones2", [1, 128], F16, kind="ExternalInput").ap()
    outp = nc.dram_tensor("out_p", [S, E], F32, kind="ExternalOutput").ap()

    with tile.TileContext(nc) as tc, ExitStack() as ctx:
        pp = ctx.enter_context(tc.tile_pool(name="persist", bufs=1))

        # persistent SBUF tensors
        qkT = [pp.tile([128, S], F16, name=f"qkT{m}", tag=f"qkT{m}")
               for m in range(8)]                       # f-major qk.T
        VA = [pp.tile([128, HPC * 65], F16, name=f"VA{s}", tag=f"VA{s}")
              for s in range(16)]                       # v + ones col per head
        OT = [pp.tile([128, 512], F16, name=f"OT{i}", tag=f"OT{i}")
              for i in range(16)]                       # normalized o.T
        EB = [pp.tile([128, 1408], F16, name=f"EB{h}", tag=f"EB{h}")
              for h in range(HPC)]                      # exp(bias-bound) bands
        EM = pp.tile([128, 512], F16, name="EM", tag="EM")   # blk0 causal 0/1
        B0 = pp.tile([128, 32], F32, name="B0", tag="B0")    # blk0 exp biases
        ONES = pp.tile([1, 128], F16, name="ONES", tag="ONES")
        WO = [pp.tile([128, E], F16, name=f"WO{k}", tag=f"WO{k}")
              for k in range(4)]

        with tc.tile_pool(name="phA", bufs=1) as pa, \
             tc.tile_pool(name="ppsA", bufs=8, space="PSUM") as ppsA:
            xTs = [pa.tile([128, S], F16, name=f"xTs{k}", tag=f"xTs{k}")
                   for k in range(8)]
            wqks = [pa.tile([128, 1024], F16, name=f"wqks{k}", tag=f"wqks{k}")
                    for k in range(8)]
            wvs = [pa.tile([128, 512], F16, name=f"wvs{k}", tag=f"wvs{k}")
                   for k in range(8)]
            for k in range(8):
                nc.sync.dma_start(xTs[k][:], xT[128 * k:128 * (k + 1), :])
                nc.sync.dma_start(wqks[k][:], wqk[128 * k:128 * (k + 1), :])
                nc.sync.dma_start(wvs[k][:], wv[128 * k:128 * (k + 1), :])
            # persistent loads (needed later than phase-A tiles)
            for h in range(HPC):
                nc.sync.dma_start(EB[h][:], ebig[h])
            nc.sync.dma_start(EM[:], em0[:])
            nc.sync.dma_start(B0[:], b0v[:])
            for k in range(4):
                nc.sync.dma_start(WO[k][:], wo[128 * k:128 * (k + 1), :])
            nc.sync.dma_start(ONES[:], ones2[:])
            for st in range(16):
                nc.gpsimd.memset(VA[st][:], 1.0)

            # ---- projection a: qkT[f, s] = (w_qk.T @ x.T) ----
            for mt in range(8):
                ps = [ppsA.tile([128, 512], F32, name=f"pa{mt}_{sc}",
                                tag="projps") for sc in range(4)]
                for kt in range(8):
                    for sc in range(4):
                        nc.tensor.matmul(
                            ps[sc][:],
                            wqks[kt][:, 128 * mt:128 * (mt + 1)],
                            xTs[kt][:, 512 * sc:512 * (sc + 1)],
                            start=(kt == 0), stop=(kt == 7))
                for sc in range(4):
                    nc.scalar.activation(qkT[mt][:, 512 * sc:512 * (sc + 1)],
                                         ps[sc][:], AF.Copy)

            # ---- projection b: v[s, f] into VA (ones column preserved) ----
            for st in range(16):
                pv = ppsA.tile([128, 512], F32, name=f"pv{st}", tag="projps")
                for kt in range(8):
                    nc.tensor.matmul(
                        pv[:],
                        xTs[kt][:, 128 * st:128 * (st + 1)],
                        wvs[kt][:],
                        start=(kt == 0), stop=(kt == 7))
                src = pv.rearrange("p (h c) -> p h c", h=HPC)
                dst = VA[st].rearrange("p (h c) -> p h c", h=HPC)
                nc.scalar.activation(dst[:, :, 0:64], src[:], AF.Copy)

        # ---- attention ----
        with tc.tile_pool(name="Pp", bufs=18) as Ppool, \
             tc.tile_pool(name="r2p", bufs=4) as r2p, \
             tc.tile_pool(name="scps", bufs=4, space="PSUM") as scps, \
             tc.tile_pool(name="ops", bufs=3, space="PSUM") as ops, \
             tc.tile_pool(name="r2ps", bufs=1, space="PSUM") as r2ps:
            for hp in range(4):
                for blk in range(4):
                    jts = list(range(8)) if blk > 0 else [4, 5, 6, 7]
                    first_jt = 3 if blk > 0 else 4
                    pv_order = [first_jt] + [j for j in jts if j != first_jt]
                    O2 = [ops.tile([65, 512], F32, name=f"O{hp}_{blk}_{par}",
                                   tag="Opair") for par in (0, 1)]
                    Pt = {}
                    for jt in jts:
                        q0, w = _qrange(jt)
                        gsb = (blk - 1) * 512 + 128 * jt
                        for par in (0, 1):
                            h = 2 * hp + par
                            Sps = scps.tile([128, 512], F32,
                                            name=f"S{hp}_{blk}_{jt}_{par}",
                                            tag="S")
                            nc.tensor.matmul(
                                Sps[:, 0:w],
                                qkT[4 + hp][64 * par:64 * par + 64,
                                            gsb:gsb + 128],
                                qkT[hp][64 * par:64 * par + 64,
                                        512 * blk + q0:512 * blk + q0 + w],
                                start=True, stop=True,
                                tile_position=(64 * par, 0))
                            P = Ppool.tile([128, 512], F16,
                                           name=f"P{hp}_{blk}_{jt}_{par}",
                                           tag="P")
                            if blk > 0:
                                nc.scalar.activation(P[:, 0:w], Sps[:, 0:w],
                                                     AF.Exp)
                                c0 = q0 - 128 * jt + 896
                                nc.vector.tensor_tensor(
                                    P[:, 0:w], P[:, 0:w],
                                    EB[h][:, c0:c0 + w], ALU.mult)
                            else:
                                idx = h * 4 + (jt - 4)
                                nc.scalar.activation(
                                    P[:, 0:w], Sps[:, 0:w], AF.Exp,
                                    bias=B0[:, idx:idx + 1])
                                nc.vector.tensor_tensor(
                                    P[:, 0:w], P[:, 0:w],
                                    EM[:, 0:w], ALU.mult)
                            Pt[(jt, par)] = (P, q0, w)
                    for i, jt in enumerate(pv_order):
                        st = 4 * (blk - 1) + jt
                        for par in (0, 1):
                            P, q0, w = Pt[(jt, par)]
                            hl = 2 * hp + par
                            nc.tensor.matmul(
                                O2[par][:, q0:q0 + w],
                                VA[st][:, 65 * hl:65 * hl + 65],
                                P[:, 0:w],
                                start=(i == 0), stop=(i == len(pv_order) - 1),
                                skip_group_check=True)
                    # normalize: o = num / denom via K=1 broadcast matmuls
                    R2 = r2ps.tile([128, 512], F32, name=f"R2_{hp}_{blk}",
                                   tag="R2")
                    for par in (0, 1):
                        rr = r2p.tile([1, 512], F32,
                                      name=f"r2_{hp}_{blk}_{par}",
                                      tag=f"r2_{par}")
                        nc.vector.reciprocal(rr[:], O2[par][64:65, :])
                        rh = r2p.tile([1, 512], F16,
                                      name=f"r2h_{hp}_{blk}_{par}",
                                      tag=f"r2h_{par}")
                        nc.scalar.activation(rh[:], rr[:], AF.Copy)
                        nc.tensor.matmul(R2[64 * par:64 * par + 64, :],
                                         ONES[0:1, 64 * par:64 * par + 64],
                                         rh[:], start=True, stop=True,
                                         tile_position=(0, 64 * par),
                                         skip_group_check=True)
                    R2s = r2p.tile([128, 512], F32, name=f"R2s_{hp}_{blk}",
                                   tag="R2s")
                    nc.scalar.activation(R2s[:], R2[:], AF.Copy)
                    ot = OT[4 * hp + blk]
                    for par in (0, 1):
                        nc.vector.tensor_tensor(
                            ot[64 * par:64 * par + 64, :],
                            O2[par][0:64, :],
                            R2s[64 * par:64 * par + 64, :], ALU.mult)

        # ---- out projection (partial over this core's head columns) ----
        with tc.tile_pool(name="phC", bufs=4) as pc, \
             tc.tile_pool(name="cps", bufs=4, space="PSUM") as cps:
            for st in range(16):
                blk, qq = st // 4, st % 4
                for half in range(2):
                    po = cps.tile([128, 512], F32, name=f"po{st}_{half}",
                                  tag="po")
                    for kt in range(4):
                        nc.tensor.matmul(
                            po[:],
                            OT[4 * kt + blk][:, 128 * qq:128 * (qq + 1)],
                            WO[kt][:, 512 * half:512 * (half + 1)],
                            start=(kt == 0), stop=(kt == 3))
                    stg = pc.tile([128, 512], F32, name=f"stg{st}_{half}",
                                  tag="stg")
                    nc.scalar.activation(stg[:], po[:], AF.Copy)
                    nc.sync.dma_start(
                        outp[128 * st:128 * (st + 1),
                             512 * half:512 * (half + 1)], stg[:])

    nc.compile()
    return nc


_NC = None


def _get_nc():
    global _NC
    if _NC is None:
        _NC = _build_nc()
    return _NC


def _host_consts():
    slopes = np.exp2(-(np.arange(H, dtype=np.float64) + 1.0) * 8.0 / H)
    p = np.arange(128)[:, None]
    c = np.arange(1408)[None, :]
    delta = (c - p - 384).astype(np.float64)
    valid = (delta >= 0) & (delta <= 512)
    eb = np.zeros((H, 128, 1408), np.float16)
    for h in range(H):
        vals = np.exp(slopes[h] * (delta - 512.0) - CM)
        eb[h] = np.where(valid, vals, 0.0).astype(np.float16)
    cc = np.arange(512)[None, :]
    em0 = (cc >= p).astype(np.float16)
    b0 = np.zeros((2, 128, 32), np.float32)  # per head-group
    for g in range(2):
        for hl in range(HPC):
            for jtl in range(4):
                b0[g, :, hl * 4 + jtl] = (
                    -slopes[8 * g + hl] * (128.0 * jtl + p[:, 0]) - CM)
    ones2 = np.ones((1, 128), np.float16)
    return slopes, eb, em0, b0, ones2


def kernel(x, w_in, w_out):
    global LAST_RESULTS
    x = np.asarray(x, dtype=np.float32)
    w_in = np.asarray(w_in, dtype=np.float32)
    w_out = np.asarray(w_out, dtype=np.float32)

    nc = _get_nc()
    _, eb, em0, b0, ones2 = _host_consts()

    in_maps = []
    for core in range(NCORES):
        b, g = divmod(core, 2)
        r0 = 512 * g
        w_qk = np.ascontiguousarray(np.concatenate(
            [w_in[r0:r0 + 512] * 0.125,
             w_in[E + r0:E + r0 + 512]], axis=0).T).astype(np.float16)
        w_v = np.ascontiguousarray(
            w_in[2 * E + r0:2 * E + r0 + 512].T).astype(np.float16)
        w_o = np.ascontiguousarray(
            w_out[:, r0:r0 + 512].T).astype(np.float16)
        xTc = np.ascontiguousarray(x[b].T).astype(np.float16)
        in_maps.append({
            "xT": xTc,
            "w_qk": w_qk,
            "w_v": w_v,
            "w_o": w_o,
            "expbig": np.ascontiguousarray(eb[8 * g:8 * g + 8]),
            "em0": em0,
            "b0v": np.ascontiguousarray(b0[g]),
            "ones2": ones2,
        })

    res = run_bass_kernel_spmd(nc, in_maps, core_ids=list(range(NCORES)))
    LAST_RESULTS = res
    out = np.stack([
        res.results[2 * b]["out_p"] + res.results[2 * b + 1]["out_p"]
        for b in range(B)
    ]).astype(np.float32)
    return out


# revision 27
# speedup vs baseline: 1.3275x; 1.3275x over previous
"""Trainium2 Bass kernel: sliding-window multihead attention w/ ALiBi.

Computation (per reference):
  qkv = x @ w_in.T ; q,k,v heads ; blocked sliding-window causal attention
  (window=512, ALiBi bias slope_h*(q_idx-kv_idx)) ; out = o @ w_out.T

Sharding: 8 cores = 4 batches x 2 head-groups (8 heads each). Each core
computes its batch's QKV for its heads, attention, and a partial out-proj
over its heads' columns. Host sums the two head-group partials per batch.

Softmax trick: P = exp(s_raw) * EXPBIG where EXPBIG = exp(bias - bound)
is a host-precomputed Toeplitz band (exact 0 outside the valid window).
The row-max subtraction is replaced by a static bound folded into EXPBIG
(block 0 uses a per-partition ACT bias instead). The softmax denominator
comes from an appended ones-column in the V matmul; normalization uses a
K=2 broadcast matmul + vector reciprocal.
"""

import os
import numpy as np
from contextlib import ExitStack

import concourse.bass as bass
import concourse.bacc as bacc
import concourse.tile as tile
import concourse.mybir as mybir
from concourse.bass_utils import run_bass_kernel_spmd

F16 = mybir.dt.float16
F32 = mybir.dt.float32
AF = mybir.ActivationFunctionType
ALU = mybir.AluOpType

B, S, E = 4, 2048, 1024
H, D, WIN = 16, 64, 512
NB = S // WIN          # 4 blocks
HPC = 8                # heads per core
NCORES = 8
CM = 6.0               # softmax bound safety margin

LAST_RESULTS = None


def _qrange(jt):
    # valid q-column range for scores j-tile jt (window band)
    lo = max(0, 128 * jt - 512)
    hi = min(512, 128 * jt + 128)
    return lo, hi - lo


def _build_nc():
    nc = bacc.Bacc("TRN2", target_bir_lowering=False, debug=False,
                   num_devices=NCORES)

    xT = nc.dram_tensor("xT", [E, S], F16, kind="ExternalInput").ap()
    wqk = nc.dram_tensor("w_qk", [E, 1024], F16, kind="ExternalInput").ap()
    wv = nc.dram_tensor("w_v", [E, 512], F16, kind="ExternalInput").ap()
    wo = nc.dram_tensor("w_o", [512, E], F16, kind="ExternalInput").ap()
    ebig = nc.dram_tensor("expbig", [4, 128, 2816], F16,
                          kind="ExternalInput").ap()
    em0 = nc.dram_tensor("em0", [128, 1024], F16, kind="ExternalInput").ap()
    b0v = nc.dram_tensor("b0v", [128, 32], F32, kind="ExternalInput").ap()
    outp = nc.dram_tensor("out_p", [S, E], F32, kind="ExternalOutput").ap()

    with tile.TileContext(nc) as tc, ExitStack() as ctx:
        pp = ctx.enter_context(tc.tile_pool(name="persist", bufs=1))

        # persistent SBUF tensors
        qkT = [pp.tile([128, S], F16, name=f"qkT{m}", tag=f"qkT{m}")
               for m in range(8)]                       # f-major qk.T
        VA = [pp.tile([128, HPC * 65], F16, name=f"VA{s}", tag=f"VA{s}")
              for s in range(16)]                       # v + ones col per head
        OT = [pp.tile([128, 512], F16, name=f"OT{i}", tag=f"OT{i}")
              for i in range(16)]                       # normalized o.T
        EB = [pp.tile([128, 2816], F16, name=f"EB{h}", tag=f"EB{h}")
              for h in range(4)]                  # exp(bias-bound) band pairs
        EM = pp.tile([128, 1024], F16, name="EM", tag="EM")  # blk0 causal 0/1
        B0 = pp.tile([128, 32], F32, name="B0", tag="B0")    # blk0 exp biases
        ONES = pp.tile([1, 64], F16, name="ONES", tag="ONES")
        WO = [pp.tile([128, E], F16, name=f"WO{k}", tag=f"WO{k}")
              for k in range(4)]

        with tc.tile_pool(name="phA", bufs=1) as pa, \
             tc.tile_pool(name="ppsA", bufs=8, space="PSUM") as ppsA:
            xTs = [pa.tile([128, S], F16, name=f"xTs{k}", tag=f"xTs{k}")
                   for k in range(8)]
            wqks = [pa.tile([128, 1024], F16, name=f"wqks{k}", tag=f"wqks{k}")
                    for k in range(8)]
            wvs = [pa.tile([128, 512], F16, name=f"wvs{k}", tag=f"wvs{k}")
                   for k in range(8)]
            for k in range(8):
                nc.sync.dma_start(xTs[k][:], xT[128 * k:128 * (k + 1), :])
                nc.sync.dma_start(wqks[k][:], wqk[128 * k:128 * (k + 1), :])
                nc.sync.dma_start(wvs[k][:], wv[128 * k:128 * (k + 1), :])
            # persistent loads (needed later than phase-A tiles)
            for h in range(4):
                nc.sync.dma_start(EB[h][:], ebig[h])
            nc.sync.dma_start(EM[:], em0[:])
            nc.sync.dma_start(B0[:], b0v[:])
            for k in range(4):
                nc.sync.dma_start(WO[k][:], wo[128 * k:128 * (k + 1), :])
            for st in range(16):
                nc.gpsimd.memset(VA[st][:], 1.0)
            nc.gpsimd.memset(ONES[:], 1.0)

            # ---- projection a: qkT[f, s] = (w_qk.T @ x.T) ----
            for mt in range(8):
                ps = [ppsA.tile([128, 512], F32, name=f"pa{mt}_{sc}",
                                tag="projps") for sc in range(4)]
                for kt in range(8):
                    for sc in range(4):
                        nc.tensor.matmul(
                            ps[sc][:],
                            wqks[kt][:, 128 * mt:128 * (mt + 1)],
                            xTs[kt][:, 512 * sc:512 * (sc + 1)],
                            start=(kt == 0), stop=(kt == 7))
                for sc in range(4):
                    nc.scalar.activation(qkT[mt][:, 512 * sc:512 * (sc + 1)],
                                         ps[sc][:], AF.Copy)

            # ---- projection b: v[s, f] into VA (ones column preserved) ----
            for st in range(16):
                pv = ppsA.tile([128, 512], F32, name=f"pv{st}", tag="projps")
                for kt in range(8):
                    nc.tensor.matmul(
                        pv[:],
                        xTs[kt][:, 128 * st:128 * (st + 1)],
                        wvs[kt][:],
                        start=(kt == 0), stop=(kt == 7))
                src = pv.rearrange("p (h c) -> p h c", h=HPC)
                dst = VA[st].rearrange("p (h c) -> p h c", h=HPC)
                nc.scalar.activation(dst[:, :, 0:64], src[:], AF.Copy)

        # ---- attention ----
        with tc.tile_pool(name="Pp", bufs=10) as Ppool, \
             tc.tile_pool(name="r2p", bufs=4) as r2p, \
             tc.tile_pool(name="scps", bufs=2, space="PSUM") as scps, \
             tc.tile_pool(name="ops", bufs=2, space="PSUM") as ops:
            for hp in range(4):
                for blk in range(4):
                    jts = list(range(8)) if blk > 0 else [4, 5, 6, 7]
                    first_jt = 3 if blk > 0 else 4
                    pv_order = [first_jt] + [j for j in jts if j != first_jt]
                    # paired psum: cols [0:512) head 2hp, [512:1024) head 2hp+1
                    # rows 0-63: o numerator, row 64: denom,
                    # rows 64-127 later overwritten by denom-recip broadcast
                    Op = ops.tile([128, 1024], F32, name=f"O{hp}_{blk}",
                                  tag="Opair")
                    Pt = {}
                    for jt in jts:
                        q0, w = _qrange(jt)
                        gsb = (blk - 1) * 512 + 128 * jt
                        Sp = scps.tile([128, 1024], F32,
                                       name=f"S{hp}_{blk}_{jt}", tag="S")
                        for par in (0, 1):
                            nc.tensor.matmul(
                                Sp[:, 512 * par:512 * par + w],
                                qkT[4 + hp][64 * par:64 * par + 64,
                                            gsb:gsb + 128],
                                qkT[hp][64 * par:64 * par + 64,
                                        512 * blk + q0:512 * blk + q0 + w],
                                start=True, stop=True,
                                tile_position=(64 * par, 0),
                                skip_group_check=True)
                        P = Ppool.tile([128, 1024], F16,
                                       name=f"P{hp}_{blk}_{jt}", tag="P")
                        c0 = q0 - 128 * jt + 896
                        for par in (0, 1):
                            if blk > 0:
                                nc.scalar.activation(
                                    P[:, 512 * par:512 * par + w],
                                    Sp[:, 512 * par:512 * par + w], AF.Exp)
                                nc.vector.tensor_tensor(
                                    P[:, 512 * par:512 * par + w],
                                    P[:, 512 * par:512 * par + w],
                                    EB[hp][:, 1408 * par + c0:
                                           1408 * par + c0 + w], ALU.mult)
                            else:
                                idx = (2 * hp + par) * 4 + (jt - 4)
                                nc.scalar.activation(
                                    P[:, 512 * par:512 * par + w],
                                    Sp[:, 512 * par:512 * par + w], AF.Exp,
                                    bias=B0[:, idx:idx + 1])
                                nc.vector.tensor_tensor(
                                    P[:, 512 * par:512 * par + w],
                                    P[:, 512 * par:512 * par + w],
                                    EM[:, 512 * par:512 * par + w], ALU.mult)
                        Pt[jt] = (P, q0, w)
                    for i, jt in enumerate(pv_order):
                        st = 4 * (blk - 1) + jt
                        for par in (0, 1):
                            P, q0, w = Pt[jt]
                            hl = 2 * hp + par
                            nc.tensor.matmul(
                                Op[0:65, 512 * par + q0:512 * par + q0 + w],
                                VA[st][:, 65 * hl:65 * hl + 65],
                                P[:, 512 * par:512 * par + w],
                                start=(i == 0), stop=(i == len(pv_order) - 1),
                                skip_group_check=True)
                    # normalize: reciprocal of denom row, gpsimd broadcast
                    # approx_fast's bitwise path misreads accumulated PSUM
                    # (non-IEEE accumulator bits) — bounce via SBUF first
                    dnc = r2p.tile([1, 1024], F32, name=f"dnc{hp}_{blk}",
                                   tag="dnc")
                    nc.scalar.activation(dnc[:], Op[64:65, :], AF.Copy)
                    rr = r2p.tile([1, 1024], F32, name=f"rr{hp}_{blk}",
                                  tag="rr")
                    nc.vector.reciprocal_approx_fast(rr[:], dnc[:])
                    rh = r2p.tile([1, 1024], F16, name=f"rh{hp}_{blk}",
                                  tag="rh")
                    nc.vector.tensor_copy(rh[:], rr[:])
                    # broadcast 1/denom into Op rows 64-127 (K=1 matmuls)
                    for par in (0, 1):
                        nc.tensor.matmul(
                            Op[64:128, 512 * par:512 * par + 512],
                            ONES[0:1, :],
                            rh[0:1, 512 * par:512 * par + 512],
                            start=True, stop=True,
                            tile_position=(0, 64),
                            skip_group_check=True)
                    R2s = r2p.tile([64, 1024], F32, name=f"R2s{hp}_{blk}",
                                   tag="R2s")
                    nc.scalar.activation(R2s[:], Op[64:128, :], AF.Copy)
                    ot = OT[4 * hp + blk]
                    for par in (0, 1):
                        nc.vector.tensor_tensor(
                            ot[64 * par:64 * par + 64, :],
                            Op[0:64, 512 * par:512 * par + 512],
                            R2s[0:64, 512 * par:512 * par + 512], ALU.mult)

        # ---- out projection (partial over this core's head columns) ----
        with tc.tile_pool(name="phC", bufs=4) as pc, \
             tc.tile_pool(name="cps", bufs=4, space="PSUM") as cps:
            for st in range(16):
                blk, qq = st // 4, st % 4
                for half in range(2):
                    po = cps.tile([128, 512], F32, name=f"po{st}_{half}",
                                  tag="po")
                    for kt in range(4):
                        nc.tensor.matmul(
                            po[:],
                            OT[4 * kt + blk][:, 128 * qq:128 * (qq + 1)],
                            WO[kt][:, 512 * half:512 * (half + 1)],
                            start=(kt == 0), stop=(kt == 3))
                    stg = pc.tile([128, 512], F32, name=f"stg{st}_{half}",
                                  tag="stg")
                    nc.scalar.activation(stg[:], po[:], AF.Copy)
                    nc.sync.dma_start(
                        outp[128 * st:128 * (st + 1),
                             512 * half:512 * (half + 1)], stg[:])

    nc.compile()
    return nc


_NC = None


def _get_nc():
    global _NC
    if _NC is None:
        _NC = _build_nc()
    return _NC


def _host_consts():
    slopes = np.exp2(-(np.arange(H, dtype=np.float64) + 1.0) * 8.0 / H)
    p = np.arange(128)[:, None]
    c = np.arange(1408)[None, :]
    delta = (c - p - 384).astype(np.float64)
    valid = (delta >= 0) & (delta <= 512)
    eb = np.zeros((H, 128, 1408), np.float16)
    for h in range(H):
        vals = np.exp(slopes[h] * (delta - 512.0) - CM)
        eb[h] = np.where(valid, vals, 0.0).astype(np.float16)
    cc = np.arange(512)[None, :]
    em0 = (cc >= p).astype(np.float16)
    em0 = np.concatenate([em0, em0], axis=1)  # paired [128, 1024]
    # pair-interleaved bands: [g, hp, 128, 2*1408]
    ebp = np.zeros((2, 4, 128, 2816), np.float16)
    for g in range(2):
        for hp in range(4):
            ebp[g, hp, :, 0:1408] = eb[8 * g + 2 * hp]
            ebp[g, hp, :, 1408:2816] = eb[8 * g + 2 * hp + 1]
    b0 = np.zeros((2, 128, 32), np.float32)  # per head-group
    for g in range(2):
        for hl in range(HPC):
            for jtl in range(4):
                b0[g, :, hl * 4 + jtl] = (
                    -slopes[8 * g + hl] * (128.0 * jtl + p[:, 0]) - CM)
    return slopes, ebp, em0, b0


def kernel(x, w_in, w_out):
    global LAST_RESULTS
    x = np.asarray(x, dtype=np.float32)
    w_in = np.asarray(w_in, dtype=np.float32)
    w_out = np.asarray(w_out, dtype=np.float32)

    nc = _get_nc()
    _, ebp, em0, b0 = _host_consts()

    in_maps = []
    for core in range(NCORES):
        b, g = divmod(core, 2)
        r0 = 512 * g
        w_qk = np.ascontiguousarray(np.concatenate(
            [w_in[r0:r0 + 512] * 0.125,
             w_in[E + r0:E + r0 + 512]], axis=0).T).astype(np.float16)
        w_v = np.ascontiguousarray(
            w_in[2 * E + r0:2 * E + r0 + 512].T).astype(np.float16)
        w_o = np.ascontiguousarray(
            w_out[:, r0:r0 + 512].T).astype(np.float16)
        xTc = np.ascontiguousarray(x[b].T).astype(np.float16)
        in_maps.append({
            "xT": xTc,
            "w_qk": w_qk,
            "w_v": w_v,
            "w_o": w_o,
            "expbig": np.ascontiguousarray(ebp[g]),
            "em0": em0,
            "b0v": np.ascontiguousarray(b0[g]),
        })

    res = run_bass_kernel_spmd(nc, in_maps, core_ids=list(range(NCORES)))
    LAST_RESULTS = res
    out = np.stack([
        res.results[2 * b]["out_p"] + res.results[2 * b + 1]["out_p"]
        for b in range(B)
    ]).astype(np.float32)
    return out
